# revision 8
# baseline (speedup 1.0000x reference)
"""Bass/Trainium2 kernel for nn_BeMultiHeadAttention (B=2, S=2048, D=1024, H=16, HD=64).

Sharding: data-parallel over tokens. 8 cores; core c handles batch b=c//4 and
query slice q0=(c%4)*512 .. +512. Each core computes K/V projections for its
full batch (2048 keys), Q projection for its 512 queries, transposed-scores
flash attention (no max subtraction needed: |score/8| <~ 2), and the output
projection for its 512 tokens. No collectives; the host concatenates shards.

v2 layout notes:
 - The K-projection bias is key-independent inside softmax (adds bk.q to every
   key's score), so it cancels exactly and is dropped entirely.
 - ACT runs PURE exp (128 x [128,1024] chunks). The softmax reciprocal moved
   to DVE reciprocal_approx_fast; the per-query broadcast stays a rank-1 PE
   matmul into the shared proj PSUM region.
 - PSUM (8 banks): scores slots 2x[128,1024] (4) + potA/potB [65,512] (2,
   single-buffered) + proj/epilogue region [128,1024] (2). The scores slots
   are claimed ONLY by scores chunks -> pure 2-deep alternation, so
   scores(c+2) waits exp(c) which ended a full chunk earlier: ACT never gaps
   on the slot rotation (the baseline lost ~535ns on most chunks to fillers
   flipping the claim parity).
 - Single-buffered pots work because the epilogue copies pots to SBUF (f32)
   immediately at pair start, releasing the banks by chunk ~1.5; the next
   pair's OT emission is deferred to chunks 4..8 (the 6-deep at pool absorbs
   the lag) so the in-order PE queue never blocks on the pot WAR.
 - Projections for pair p+1 and the epilogue of pair p-1 serialize through
   the proj region (claims >= 2 chunks apart so each claim's WAR on the
   previous claim's DVE reader is already satisfied at issue).
 - DMA: consolidated big-line transfers (xt per pair = 4KB/partition lines),
   critical pair-0 set first in first-use order.
 - Output bias via a K=1 matmul (ones row x bo2) appended to the y
   accumulation; bo2 = bv @ Wo + bo folds the V bias.
"""

import numpy as np
import ml_dtypes

import concourse.bass as bass
import concourse.tile as tile
import concourse.mybir as mybir
from concourse.bass_utils import run_bass_kernel_spmd


BF16 = ml_dtypes.bfloat16

B, S, D, H, HD = 2, 2048, 1024, 16, 64
NCORES = 8
QS = S * B // NCORES          # 512 queries per core
NPAIR = H // 2                # 8 head pairs
NKC = S // 128                # 16 key chunks
SCALE = 1.0 / np.sqrt(HD)     # 0.125

_bf = mybir.dt.bfloat16
_f32 = mybir.dt.float32


def _split_excess_waits(nc, max_waits=1):
    """This container's walrus only accepts one sync-wait per instruction;
    split extras onto preceding NoOps on the same engine."""
    for fn in nc.m.functions:
        for bb in fn.blocks:
            new_insts = []
            for inst in bb.instructions:
                si = inst.sync_info
                if si is not None and si.on_wait and len(si.on_wait) > max_waits:
                    waits = list(si.on_wait)
                    extra, keep = waits[:-max_waits], waits[-max_waits:]
                    while extra:
                        chunk, extra = extra[:max_waits], extra[max_waits:]
                        new_insts.append(mybir.InstNoOp(
                            name=nc.get_next_instruction_name(),
                            engine=inst.engine,
                            sync_info=mybir.SyncInfo(on_wait=chunk, on_update=[]),
                            bass_nofuse=True))
                    inst.sync_info = mybir.SyncInfo(
                        on_wait=keep, on_update=list(si.on_update))
                new_insts.append(inst)
            bb.instructions = new_insts


def build_nc():
    nc = bass.Bass("TRN2", target_bir_lowering=False, debug=False)

    xt_in = nc.declare_dram_parameter("xt", [128, 8, S], _bf, isOutput=False)
    xtq_in = nc.declare_dram_parameter("xtq", [128, 8, QS], _bf, isOutput=False)
    wk_in = nc.declare_dram_parameter("wk", [128, NPAIR * 128], _bf, isOutput=False)
    wq_in = nc.declare_dram_parameter("wq", [128, NPAIR * 128], _bf, isOutput=False)
    wv_in = nc.declare_dram_parameter("wv", [128, NPAIR * 128], _bf, isOutput=False)
    bq_in = nc.declare_dram_parameter("bq", [128, NPAIR], _f32, isOutput=False)
    wo_in = nc.declare_dram_parameter("wo", [128, 8, D], _bf, isOutput=False)
    bo_in = nc.declare_dram_parameter("bo", [1, D], _bf, isOutput=False)
    id_in = nc.declare_dram_parameter("ident", [64, 64], _bf, isOutput=False)
    # bf16 output halves the output-DMA drain at the tail; the host upcasts.
    out_d = nc.declare_dram_parameter("out", [QS, D], _bf, isOutput=True)

    Exp = mybir.ActivationFunctionType.Exp

    with tile.TileContext(nc) as tc:
        with (
            tc.tile_pool(name="singles", bufs=1) as singles,
            tc.tile_pool(name="attn", bufs=6) as attn_pool,
            tc.tile_pool(name="ep", bufs=2) as ep_pool,
            tc.tile_pool(name="ysb", bufs=2) as y_pool,
        ):
            ones_bf = singles.tile([1, 128], _bf)
            nc.vector.memset(ones_bf[:], 1.0)
            warm_rhs = singles.tile([1, 512], _bf)
            nc.vector.memset(warm_rhs[:], 1.0)

            wk_sb = singles.tile([128, NPAIR * 128], _bf)
            wq_sb = singles.tile([128, NPAIR * 128], _bf)
            wv_sb = singles.tile([128, NPAIR * 128], _bf)
            bq_sb = singles.tile([128, NPAIR], _f32)
            bo_sb = singles.tile([1, D], _bf)
            id_sb = singles.tile([64, 64], _bf)
            xtq_sb = singles.tile([128, 8, QS], _bf)
            xt_sb = singles.tile([128, 8, S], _bf)
            wo_sb = singles.tile([128, 8, D], _bf)

            # --- DMA emission: pair-0 critical set first, in first-use order
            # (kt g0 needs wk + xt cols 0:1024; qt needs wq + bq + xtq p0).
            # Full-tensor weight DMAs get 2KB/partition lines; per-pair xt
            # gets 4KB lines. Everything in as few big transfers as possible.
            nc.sync.dma_start(wq_sb[:], wq_in[:])
            nc.sync.dma_start(bq_sb[:], bq_in[:])
            nc.sync.dma_start(xtq_sb[:, 0, :], xtq_in[:, 0, :])
            nc.sync.dma_start(wk_sb[:], wk_in[:])
            nc.sync.dma_start(xt_sb[:, 0, 0:1024], xt_in[:, 0, 0:1024])
            nc.sync.dma_start(id_sb[:], id_in[:])
            nc.sync.dma_start(xt_sb[:, 0, 1024:2048], xt_in[:, 0, 1024:2048])
            nc.sync.dma_start(wv_sb[:], wv_in[:])
            for p in range(1, NPAIR):
                nc.sync.dma_start(xt_sb[:, p, :], xt_in[:, p, :])
                nc.sync.dma_start(xtq_sb[:, p, :], xtq_in[:, p, :])
            nc.sync.dma_start(bo_sb[:], bo_in[:])
            nc.sync.dma_start(wo_sb[:], wo_in[:])

            kt_sb = singles.tile([128, NPAIR, S], _bf)
            qt_sb = singles.tile([128, NPAIR, QS], _bf)
            # V layout per (pair, keychunk): [V_A(64) | ones | V_B(64) | ones]
            v_sb = singles.tile([128, NPAIR, NKC, 130], _bf)
            # only the two ones-columns need the memset; 130 = 2*65 so the
            # ones-columns form a uniform stride-65 pattern (3-dim AP)
            nc.vector.memset(
                v_sb.rearrange("p a c (h e) -> p (a c h) e", e=65)[:, :, 64:65],
                1.0)

            otn = [singles.tile([128, QS], _bf, name=f"otn{p}") for p in range(NPAIR)]

            # epilogue SBUF scratch (per pair, double-buffered)
            def ep_tiles():
                return dict(
                    sums=ep_pool.tile([1, 2 * QS], _f32, tag="sums",
                                      name="sums"),
                    nr_t=ep_pool.tile([1, 2 * QS], _f32, tag="nr_t",
                                      name="nr_t"),
                    nr_u=ep_pool.tile([1, 2 * QS], _f32, tag="nr_u",
                                      name="nr_u"),
                    recipb=ep_pool.tile([1, 2 * QS], _bf, tag="recipb",
                                        name="recipb"),
                    bcast=ep_pool.tile([64, 2 * QS], _f32, tag="bcast",
                                       name="bcast"),
                    pca=ep_pool.tile([64, QS], _f32, tag="pca", name="pca"),
                    pcb=ep_pool.tile([64, QS], _f32, tag="pcb", name="pcb"),
                    tmpb=ep_pool.tile([64, QS], _bf, tag="tmpb", name="tmpb"),
                )

            # PSUM: scores 2x[128,1024] (4 banks, scores-only claims) +
            # potA/potB [65,512] single-buffered (2 banks) + proj [128,1024]
            # (2 banks, serialized claims for projections/epilogue/tail-y).
            with (
                tc.tile_pool(name="pslot", bufs=2, space="PSUM") as slot_pool,
                tc.tile_pool(name="ppot", bufs=1, space="PSUM") as pot_pool,
                tc.tile_pool(name="pproj", bufs=1, space="PSUM") as proj_pool,
            ):
                def slot(nm):
                    return slot_pool.tile([128, 1024], _f32, tag="slot", name=nm)

                def proj(nm):
                    return proj_pool.tile([128, 1024], _f32, tag="proj", name=nm)

                # PE warm-up: dummy matmuls (dep only on memsets) ramp the HAM
                # clock gate while the pair-0 DMAs land.
                wps = slot("warm")
                for i in range(8):
                    nc.tensor.matmul(wps[:, 0:512], ones_bf[:], warm_rhs[:],
                                     start=True, stop=True)

                # ---- projection units (each claims the proj region once) ----
                def emit_kt(p, g):
                    """kt for pair p, keys [g*1024, (g+1)*1024): 2 MMs + cast."""
                    ws = slice(p * 128, (p + 1) * 128)
                    ps = proj(f"kt{p}_{g}")
                    for i in range(2):
                        t0 = g * 1024 + i * 512
                        nc.tensor.matmul(
                            ps[:, i * 512:(i + 1) * 512],
                            wk_sb[:, ws],
                            xt_sb[:, p, t0:t0 + 512],
                            start=True, stop=True)
                    nc.vector.tensor_copy(
                        kt_sb[:, p, g * 1024:(g + 1) * 1024], ps[:])

                def emit_qt(p):
                    psq = proj(f"qt{p}")
                    ws = slice(p * 128, (p + 1) * 128)
                    nc.tensor.matmul(psq[:, 0:QS], wq_sb[:, ws], xtq_sb[:, p, :],
                                     start=True, stop=True)
                    nc.vector.tensor_scalar_add(
                        qt_sb[:, p, :], psq[:, 0:QS], bq_sb[:, p:p + 1])

                def emit_v(p, g):
                    """V for pair p, key chunks [g*8, g*8+8): 8 MMs + cast."""
                    ws = slice(p * 128, (p + 1) * 128)
                    psv = proj(f"v{p}_{g}")
                    psv8 = psv.rearrange("p (c e) -> p c e", e=128)
                    for i in range(8):
                        c = g * 8 + i
                        nc.tensor.matmul(
                            psv8[:, i, :],
                            xt_sb[:, p, c * 128:(c + 1) * 128],
                            wv_sb[:, ws],
                            start=True, stop=True)
                    dst = v_sb[:, p, g * 8:(g + 1) * 8, :].rearrange(
                        "p c (h e) -> p c h e", e=65)[:, :, :, 0:64]
                    src = psv[:, 0:1024].rearrange(
                        "p (c h e) -> p c h e", h=2, e=64)
                    nc.vector.tensor_copy(dst, src)

                # ---- epilogue for pair p as staged thunks ----
                def emit_epilogue_stages(p, pots):
                    t = ep_tiles()

                    def s_sums():
                        # softmax sums (f32) + unnormalized outputs to SBUF;
                        # pots are fully released after these reads.
                        for a, pc in ((0, t["pca"]), (1, t["pcb"])):
                            nc.vector.tensor_copy(
                                t["sums"][0:1, a * QS:(a + 1) * QS],
                                pots[a][64:65, :])
                            nc.vector.tensor_copy(pc[:], pots[a][0:64, :])

                    def s_recip():
                        # Newton-Raphson 1/Z on the (otherwise idle) GpSimd
                        # engine. Z = softmax sum over 2048 keys is tightly
                        # concentrated around ~2170 for ~N(0,1) inputs, so a
                        # constant seed converges; 3 iterations tolerate a
                        # seed off by up to ~40% (err ~ e0^8). Sign-invariant
                        # form: u = -r,  u' = (Z*u + 2)*u  keeps one fused
                        # scalar_tensor_tensor per iteration.
                        Z0 = 2175.0
                        nc.gpsimd.memset(t["nr_u"][0:1, :], -1.0 / Z0)
                        for _ in range(3):
                            nc.gpsimd.tensor_tensor(
                                t["nr_t"][0:1, :], t["sums"][0:1, :],
                                t["nr_u"][0:1, :], op=mybir.AluOpType.mult)
                            nc.gpsimd.tensor_scalar(
                                t["nr_t"][0:1, :], t["nr_t"][0:1, :],
                                -2.0, None, op0=mybir.AluOpType.subtract)
                            nc.gpsimd.tensor_tensor(
                                t["nr_u"][0:1, :], t["nr_t"][0:1, :],
                                t["nr_u"][0:1, :], op=mybir.AluOpType.mult)
                        nc.gpsimd.tensor_scalar_mul(t["recipb"][0:1, :],
                                                    t["nr_u"][0:1, :], -1.0)

                    def s_bcast_mm():
                        pb = t["pb"] = proj(f"ep{p}")
                        for a in range(2):
                            nc.tensor.matmul(
                                pb[0:64, a * QS:(a + 1) * QS],
                                ones_bf[0:1, 0:64],
                                t["recipb"][0:1, a * QS:(a + 1) * QS],
                                start=True, stop=True)

                    def s_bcast_cp():
                        nc.vector.tensor_copy(t["bcast"][:],
                                              t["pb"][0:64, 0:2 * QS])

                    def s_mul():
                        nc.vector.tensor_mul(otn[p][0:64, :], t["pca"][:],
                                             t["bcast"][:, 0:QS])
                        nc.vector.tensor_mul(t["tmpb"][:], t["pcb"][:],
                                             t["bcast"][:, QS:2 * QS])

                    def s_shift():
                        ps2 = t["ps2"] = proj(f"sh{p}")
                        nc.tensor.matmul(ps2[64:128, 0:QS], id_sb[:],
                                         t["tmpb"][:], start=True, stop=True,
                                         tile_position=(0, 64))

                    def s_ocp():
                        nc.vector.tensor_copy(otn[p][64:128, :],
                                              t["ps2"][64:128, 0:QS])

                    return [s_sums, s_recip, s_bcast_mm, s_bcast_cp,
                            s_mul, s_shift, s_ocp]

                prev_ep = {"p": None, "pots": None}

                def emit_attn(p, fillers):
                    """fillers: dict chunk_idx -> list of thunks emitted after
                    that chunk's exp. OT emission is deferred (schedule below)
                    so single-buffered pots never block the in-order PE queue:
                    the pot WAR (prev pair's s_sums copies) resolves by chunk
                    ~1.5 and the first OT is emitted at chunk 4."""
                    pots = [pot_pool.tile([65, QS], _f32, tag=f"pot{a}",
                                          name=f"pot{p}_{a}") for a in range(2)]
                    ats_q = []
                    emitted = {"n": 0}

                    def emit_ot():
                        c = emitted["n"]
                        emitted["n"] += 1
                        at = ats_q.pop(0)
                        for a in range(2):
                            nc.tensor.matmul(
                                pots[a][:],
                                v_sb[:, p, c, 65 * a:65 * a + 65],
                                at[:, a * QS:(a + 1) * QS],
                                start=(c == 0), stop=(c == NKC - 1))

                    # chunk -> number of OTs to emit after that chunk's exp
                    ot_sched = {4: 1, 5: 2, 6: 2, 7: 2, 8: 1}

                    for c in range(NKC):
                        pss = slot(f"pss{p}_{c}")
                        for a in range(2):
                            r = slice(64 * a, 64 * a + 64)
                            nc.tensor.matmul(
                                pss[:, a * QS:(a + 1) * QS],
                                kt_sb[r, p, c * 128:(c + 1) * 128],
                                qt_sb[r, p, :],
                                start=True, stop=True)
                        at = attn_pool.tile([128, 2 * QS], _bf, tag="at")
                        nc.scalar.activation(at[:], pss[:], Exp, scale=SCALE)
                        ats_q.append(at)
                        for thunk in fillers.get(c, ()):
                            thunk()
                        if c >= 9:
                            emit_ot()
                        else:
                            for _ in range(ot_sched.get(c, 0)):
                                emit_ot()
                    prev_ep["p"], prev_ep["pots"] = p, pots
                    # OT(15) deferred to the next pair's chunk 0 so the next
                    # scores/exp start before it in PE order
                    return emit_ot

                # software pipeline: proj(0)+qt(0) upfront (during DMA);
                # proj(p+1) and the epilogue(p-1) interleave into attn(p).
                emit_kt(0, 0)
                emit_qt(0)
                pending_ot = None
                for p in range(NPAIR):
                    fillers = {}

                    def put(c, thunk):
                        fillers.setdefault(c, []).append(thunk)

                    if pending_ot is not None:
                        put(0, pending_ot)
                    # own-pair V projection: cast lands ~c2.3/c4.3, first OT
                    # (deferred to c4) reads v chunks 0.. just in time
                    put(1, lambda p=p: emit_v(p, 0))
                    put(3, lambda p=p: emit_v(p, 1))
                    if p == 0:
                        put(5, lambda: emit_kt(0, 1))
                        put(7, lambda: emit_qt(1))
                        put(9, lambda: emit_kt(1, 0))
                        put(11, lambda: emit_kt(1, 1))
                    if prev_ep["pots"] is not None:
                        stages = emit_epilogue_stages(prev_ep["p"],
                                                      prev_ep["pots"])
                        # sums@0 (releases pots), Pool NR recip@1 (latency
                        # hidden until the bcast matmul at c7), then the
                        # bcast/mul/shift tail
                        for c, s in zip((0, 1, 7, 8, 9, 10, 11), stages):
                            put(c, s)
                    if p >= 1 and p + 1 < NPAIR:
                        q = p + 1
                        put(12, lambda q=q: emit_qt(q))
                        put(14, lambda q=q: emit_kt(q, 0))
                        put(15, lambda q=q: emit_kt(q, 1))
                    pending_ot = emit_attn(p, fillers)
                pending_ot()

                groups = [(j, dh) for j in range(QS // 128) for dh in range(2)]
                pys = {}

                def y_prefix(g, npre=NPAIR - 1):
                    j, dh = g
                    dsl = slice(dh * 512, (dh + 1) * 512)
                    py = slot(f"y{j}_{dh}")
                    pys[g] = py
                    for k in range(npre):
                        nc.tensor.matmul(
                            py[:, 0:512],
                            otn[k][:, j * 128:(j + 1) * 128],
                            wo_sb[:, k, dsl],
                            start=(k == 0), stop=False)

                def y_finish(g, kfrom=NPAIR - 1):
                    j, dh = g
                    dsl = slice(dh * 512, (dh + 1) * 512)
                    py = pys[g]
                    for k in range(kfrom, NPAIR):
                        nc.tensor.matmul(
                            py[:, 0:512],
                            otn[k][:, j * 128:(j + 1) * 128],
                            wo_sb[:, k, dsl],
                            start=False, stop=False)
                    nc.tensor.matmul(py[:, 0:512], ones_bf[0:1, :],
                                     bo_sb[0:1, dsl],
                                     start=False, stop=True)
                    ysb = y_pool.tile([128, 512], _bf, tag="ysb")
                    nc.vector.tensor_copy(ysb[:], py[:, 0:512])
                    nc.sync.dma_start(
                        out_d[j * 128:(j + 1) * 128, dsl], ysb[:])

                # tail: the last pair's epilogue interleaves with group-0/1
                # prefixes (covering the Pool NR latency) and the y groups
                # pipeline 2-deep through the slot pool so the PE never
                # idles long enough to re-throttle.
                stages = emit_epilogue_stages(prev_ep["p"], prev_ep["pots"])
                s_sums, s_recip, s_bcast_mm, s_bcast_cp, s_mul, s_shift, \
                    s_ocp = stages
                s_sums(); s_recip()
                y_prefix(groups[0])
                y_prefix(groups[1])
                s_bcast_mm(); s_bcast_cp(); s_mul(); s_shift(); s_ocp()
                y_finish(groups[0])
                for gi in range(2, len(groups)):
                    y_prefix(groups[gi])
                    y_finish(groups[gi - 1])
                y_finish(groups[-1])

    _split_excess_waits(nc, 1)
    return nc


def _blockdiag_pack(w):
    """[H, HD, HD] -> [128, NPAIR*128] blockdiagonal per pair, k-major."""
    out = np.zeros((128, NPAIR * 128), np.float32)
    for p in range(NPAIR):
        out[0:64, p * 128 + 0:p * 128 + 64] = w[2 * p]
        out[64:128, p * 128 + 64:p * 128 + 128] = w[2 * p + 1]
    return out.astype(BF16)


def _bias_pack(b):
    """[H, HD] -> [128, NPAIR] (pair bias along partitions)."""
    out = np.zeros((128, NPAIR), np.float32)
    for p in range(NPAIR):
        out[0:64, p] = b[2 * p]
        out[64:128, p] = b[2 * p + 1]
    return out


def prepare_inputs(X, Wq, bq, Wk, bk, Wv, bv, Wo, bo):
    """Host-side shard + pack. Returns in_maps (one dict per core).

    bk is accepted but unused: a per-query constant added to every key's
    score cancels exactly in softmax."""
    X = np.asarray(X, np.float32)
    Wo = np.asarray(Wo, np.float32)
    # fold the V bias through the output projection: bo2 = bv @ Wo + bo
    bo2 = (np.asarray(bv, np.float32).reshape(-1) @ Wo
           + np.asarray(bo, np.float32))
    common = {
        "wk": _blockdiag_pack(np.asarray(Wk, np.float32)),
        "wq": _blockdiag_pack(np.asarray(Wq, np.float32)),
        "wv": _blockdiag_pack(np.asarray(Wv, np.float32)),
        "bq": _bias_pack(np.asarray(bq, np.float32)),
        "wo": np.ascontiguousarray(
            Wo.reshape(8, 128, D).transpose(1, 0, 2)
        ).astype(BF16),
        "bo": bo2.reshape(1, D).astype(BF16),
        "ident": np.eye(64, dtype=np.float32).astype(BF16),
    }
    xts = []
    for b in range(B):
        xt = np.ascontiguousarray(X[b].T)                   # [D, S]
        xts.append(np.ascontiguousarray(
            xt.reshape(8, 128, S).transpose(1, 0, 2)).astype(BF16))
    in_maps = []
    for c in range(NCORES):
        b = c // (NCORES // B)
        q0 = (c % (NCORES // B)) * QS
        m = dict(common)
        m["xt"] = xts[b]
        m["xtq"] = np.ascontiguousarray(xts[b][:, :, q0:q0 + QS])
        in_maps.append(m)
    return in_maps


_NC_CACHE = None


def _get_nc():
    global _NC_CACHE
    if _NC_CACHE is None:
        _NC_CACHE = build_nc()
    return _NC_CACHE


def kernel(X, Wq, bq, Wk, bk, Wv, bv, Wo, bo):
    nc = _get_nc()
    in_maps = prepare_inputs(X, Wq, bq, Wk, bk, Wv, bv, Wo, bo)
    res = run_bass_kernel_spmd(nc, in_maps, core_ids=list(range(NCORES)))
    out = np.empty((B, S, D), np.float32)
    for c in range(NCORES):
        b = c // (NCORES // B)
        q0 = (c % (NCORES // B)) * QS
        out[b, q0:q0 + QS, :] = np.asarray(res.results[c]["out"],
                                           dtype=np.float32)
    return out


# revision 13
# speedup vs baseline: 4.2454x; 4.2454x over previous
"""Bass/Trainium2 kernel for nn_BeMultiHeadAttention (B=2, S=2048, D=1024, H=16, HD=64).

Sharding: data-parallel over tokens. 8 cores; core c handles batch b=c//4 and
query slice q0=(c%4)*512 .. +512. Each core computes K/V projections for its
full batch (2048 keys), Q projection for its 512 queries, transposed-scores
flash attention (no max subtraction needed: |score/8| <~ 2), and the output
projection for its 512 tokens. No collectives; the host concatenates shards.

v2 layout notes:
 - The K-projection bias is key-independent inside softmax (adds bk.q to every
   key's score), so it cancels exactly and is dropped entirely.
 - ACT runs PURE exp (128 x [128,1024] chunks). The softmax reciprocal moved
   to DVE reciprocal_approx_fast; the per-query broadcast stays a rank-1 PE
   matmul into the shared proj PSUM region.
 - PSUM (8 banks): scores slots 2x[128,1024] (4) + potA/potB [65,512] (2,
   single-buffered) + proj/epilogue region [128,1024] (2). The scores slots
   are claimed ONLY by scores chunks -> pure 2-deep alternation, so
   scores(c+2) waits exp(c) which ended a full chunk earlier: ACT never gaps
   on the slot rotation (the baseline lost ~535ns on most chunks to fillers
   flipping the claim parity).
 - Single-buffered pots work because the epilogue copies pots to SBUF (f32)
   immediately at pair start, releasing the banks by chunk ~1.5; the next
   pair's OT emission is deferred to chunks 4..8 (the 6-deep at pool absorbs
   the lag) so the in-order PE queue never blocks on the pot WAR.
 - Projections for pair p+1 and the epilogue of pair p-1 serialize through
   the proj region (claims >= 2 chunks apart so each claim's WAR on the
   previous claim's DVE reader is already satisfied at issue).
 - DMA: consolidated big-line transfers (xt per pair = 4KB/partition lines),
   critical pair-0 set first in first-use order.
 - Output bias via a K=1 matmul (ones row x bo2) appended to the y
   accumulation; bo2 = bv @ Wo + bo folds the V bias.
"""

import numpy as np
import ml_dtypes

import concourse.bass as bass
import concourse.tile as tile
import concourse.mybir as mybir
from concourse.bass_utils import run_bass_kernel_spmd


BF16 = ml_dtypes.bfloat16

B, S, D, H, HD = 2, 2048, 1024, 16, 64
NCORES = 8
QS = S * B // NCORES          # 512 queries per core
NPAIR = H // 2                # 8 head pairs
NKC = S // 128                # 16 key chunks
SCALE = 1.0 / np.sqrt(HD)     # 0.125

_bf = mybir.dt.bfloat16
_f32 = mybir.dt.float32


def _split_excess_waits(nc, max_waits=1):
    """This container's walrus only accepts one sync-wait per instruction;
    split extras onto preceding NoOps on the same engine."""
    for fn in nc.m.functions:
        for bb in fn.blocks:
            new_insts = []
            for inst in bb.instructions:
                si = inst.sync_info
                if si is not None and si.on_wait and len(si.on_wait) > max_waits:
                    waits = list(si.on_wait)
                    extra, keep = waits[:-max_waits], waits[-max_waits:]
                    while extra:
                        chunk, extra = extra[:max_waits], extra[max_waits:]
                        new_insts.append(mybir.InstNoOp(
                            name=nc.get_next_instruction_name(),
                            engine=inst.engine,
                            sync_info=mybir.SyncInfo(on_wait=chunk, on_update=[]),
                            bass_nofuse=True))
                    inst.sync_info = mybir.SyncInfo(
                        on_wait=keep, on_update=list(si.on_update))
                new_insts.append(inst)
            bb.instructions = new_insts


def build_nc():
    nc = bass.Bass("TRN2", target_bir_lowering=False, debug=False)

    xt_in = nc.declare_dram_parameter("xt", [128, 8, S], _bf, isOutput=False)
    xtq_in = nc.declare_dram_parameter("xtq", [128, 8, QS], _bf, isOutput=False)
    wk_in = nc.declare_dram_parameter("wk", [128, NPAIR * 128], _bf, isOutput=False)
    wq_in = nc.declare_dram_parameter("wq", [128, NPAIR * 128], _bf, isOutput=False)
    wv_in = nc.declare_dram_parameter("wv", [128, NPAIR * 128], _bf, isOutput=False)
    bq_in = nc.declare_dram_parameter("bq", [128, NPAIR], _f32, isOutput=False)
    wo_in = nc.declare_dram_parameter("wo", [128, 8, D], _bf, isOutput=False)
    bo_in = nc.declare_dram_parameter("bo", [1, D], _bf, isOutput=False)
    id_in = nc.declare_dram_parameter("ident", [64, 64], _bf, isOutput=False)
    # bf16 output halves the output-DMA drain at the tail; the host upcasts.
    out_d = nc.declare_dram_parameter("out", [QS, D], _bf, isOutput=True)

    Exp = mybir.ActivationFunctionType.Exp

    with tile.TileContext(nc) as tc:
        with (
            tc.tile_pool(name="singles", bufs=1) as singles,
            tc.tile_pool(name="attn", bufs=6) as attn_pool,
            tc.tile_pool(name="ep", bufs=2) as ep_pool,
            tc.tile_pool(name="ysb", bufs=2) as y_pool,
        ):
            ones_bf = singles.tile([1, 128], _bf)
            nc.vector.memset(ones_bf[:], 1.0)
            warm_rhs = singles.tile([1, 512], _bf)
            nc.vector.memset(warm_rhs[:], 1.0)

            wk_sb = singles.tile([128, NPAIR * 128], _bf)
            wq_sb = singles.tile([128, NPAIR * 128], _bf)
            wv_sb = singles.tile([128, NPAIR * 128], _bf)
            bq_sb = singles.tile([128, NPAIR], _f32)
            bo_sb = singles.tile([1, D], _bf)
            id_sb = singles.tile([64, 64], _bf)
            xtq_sb = singles.tile([128, 8, QS], _bf)
            xt_sb = singles.tile([128, 8, S], _bf)
            wo_sb = singles.tile([128, 8, D], _bf)

            # --- DMA emission: pair-0 critical set first, in first-use order
            # (kt g0 needs wk + xt cols 0:1024; qt needs wq + bq + xtq p0).
            # Full-tensor weight DMAs get 2KB/partition lines; per-pair xt
            # gets 4KB lines. Everything in as few big transfers as possible.
            nc.sync.dma_start(wq_sb[:], wq_in[:])
            nc.sync.dma_start(bq_sb[:], bq_in[:])
            nc.sync.dma_start(xtq_sb[:, 0, :], xtq_in[:, 0, :])
            nc.sync.dma_start(wk_sb[:], wk_in[:])
            nc.sync.dma_start(xt_sb[:, 0, 0:1024], xt_in[:, 0, 0:1024])
            nc.sync.dma_start(id_sb[:], id_in[:])
            nc.sync.dma_start(xt_sb[:, 0, 1024:2048], xt_in[:, 0, 1024:2048])
            nc.sync.dma_start(wv_sb[:], wv_in[:])
            for p in range(1, NPAIR):
                nc.sync.dma_start(xt_sb[:, p, :], xt_in[:, p, :])
                nc.sync.dma_start(xtq_sb[:, p, :], xtq_in[:, p, :])
            nc.sync.dma_start(bo_sb[:], bo_in[:])
            nc.sync.dma_start(wo_sb[:], wo_in[:])

            kt_sb = singles.tile([128, NPAIR, S], _bf)
            qt_sb = singles.tile([128, NPAIR, QS], _bf)
            # V layout per (pair, keychunk): [V_A(64) | ones | V_B(64) | ones]
            v_sb = singles.tile([128, NPAIR, NKC, 130], _bf)
            # only the two ones-columns need the memset; 130 = 2*65 so the
            # ones-columns form a uniform stride-65 pattern (3-dim AP)
            nc.vector.memset(
                v_sb.rearrange("p a c (h e) -> p (a c h) e", e=65)[:, :, 64:65],
                1.0)

            otn = [singles.tile([128, QS], _bf, name=f"otn{p}") for p in range(NPAIR)]

            # epilogue SBUF scratch (per pair, double-buffered)
            def ep_tiles():
                return dict(
                    sums=ep_pool.tile([1, 2 * QS], _f32, tag="sums",
                                      name="sums"),
                    lnrow=ep_pool.tile([1, 2 * QS], _f32, tag="lnrow",
                                       name="lnrow"),
                    recipb=ep_pool.tile([1, 2 * QS], _bf, tag="recipb",
                                        name="recipb"),
                    bcast=ep_pool.tile([64, 2 * QS], _f32, tag="bcast",
                                       name="bcast"),
                    pca=ep_pool.tile([64, QS], _f32, tag="pca", name="pca"),
                    pcb=ep_pool.tile([64, QS], _f32, tag="pcb", name="pcb"),
                    tmpb=ep_pool.tile([64, QS], _bf, tag="tmpb", name="tmpb"),
                )

            # PSUM: scores 2x[128,1024] (4 banks, scores-only claims) +
            # potA/potB [65,512] single-buffered (2 banks) + proj [128,1024]
            # (2 banks, serialized claims for projections/epilogue/tail-y).
            with (
                tc.tile_pool(name="pslot", bufs=2, space="PSUM") as slot_pool,
                tc.tile_pool(name="ppot", bufs=1, space="PSUM") as pot_pool,
                tc.tile_pool(name="pproj", bufs=1, space="PSUM") as proj_pool,
            ):
                def slot(nm):
                    return slot_pool.tile([128, 1024], _f32, tag="slot", name=nm)

                def proj(nm):
                    return proj_pool.tile([128, 1024], _f32, tag="proj", name=nm)

                # PE warm-up: dummy matmuls (dep only on memsets) ramp the HAM
                # clock gate while the pair-0 DMAs land.
                wps = slot("warm")
                for i in range(8):
                    nc.tensor.matmul(wps[:, 0:512], ones_bf[:], warm_rhs[:],
                                     start=True, stop=True)

                # ---- projection units (each claims the proj region once) ----
                def emit_kt(p, g):
                    """kt for pair p, keys [g*1024, (g+1)*1024): 2 MMs + cast."""
                    ws = slice(p * 128, (p + 1) * 128)
                    ps = proj(f"kt{p}_{g}")
                    for i in range(2):
                        t0 = g * 1024 + i * 512
                        nc.tensor.matmul(
                            ps[:, i * 512:(i + 1) * 512],
                            wk_sb[:, ws],
                            xt_sb[:, p, t0:t0 + 512],
                            start=True, stop=True)
                    nc.vector.tensor_copy(
                        kt_sb[:, p, g * 1024:(g + 1) * 1024], ps[:])

                def emit_qt(p):
                    psq = proj(f"qt{p}")
                    ws = slice(p * 128, (p + 1) * 128)
                    nc.tensor.matmul(psq[:, 0:QS], wq_sb[:, ws], xtq_sb[:, p, :],
                                     start=True, stop=True)
                    nc.vector.tensor_scalar_add(
                        qt_sb[:, p, :], psq[:, 0:QS], bq_sb[:, p:p + 1])

                def emit_v(p, g):
                    """V for pair p, key chunks [g*8, g*8+8): 8 MMs + cast."""
                    ws = slice(p * 128, (p + 1) * 128)
                    psv = proj(f"v{p}_{g}")
                    psv8 = psv.rearrange("p (c e) -> p c e", e=128)
                    for i in range(8):
                        c = g * 8 + i
                        nc.tensor.matmul(
                            psv8[:, i, :],
                            xt_sb[:, p, c * 128:(c + 1) * 128],
                            wv_sb[:, ws],
                            start=True, stop=True)
                    dst = v_sb[:, p, g * 8:(g + 1) * 8, :].rearrange(
                        "p c (h e) -> p c h e", e=65)[:, :, :, 0:64]
                    src = psv[:, 0:1024].rearrange(
                        "p (c h e) -> p c h e", h=2, e=64)
                    nc.vector.tensor_copy(dst, src)

                # ---- epilogue for pair p as staged thunks ----
                def emit_epilogue_stages(p, pots):
                    t = ep_tiles()

                    def s_sums():
                        # softmax sums (f32) + unnormalized outputs to SBUF;
                        # pots are fully released after these reads.
                        for a, pc in ((0, t["pca"]), (1, t["pcb"])):
                            nc.vector.tensor_copy(
                                t["sums"][0:1, a * QS:(a + 1) * QS],
                                pots[a][64:65, :])
                            nc.vector.tensor_copy(pc[:], pots[a][0:64, :])

                    def s_ln():
                        # 1/Z as exp(-ln Z): both anchors live in the
                        # natural_log_exp_and_others table set (no reload),
                        # and [1,1024]-row transcendentals are cheapest on
                        # ACT (~1.15us each; DVE/Pool lack a usable divide).
                        nc.scalar.activation(t["lnrow"][0:1, :],
                                             t["sums"][0:1, :],
                                             mybir.ActivationFunctionType.Ln)

                    def s_recip():
                        nc.scalar.activation(t["recipb"][0:1, :],
                                             t["lnrow"][0:1, :],
                                             Exp, scale=-1.0)

                    def s_bcast_mm():
                        pb = t["pb"] = proj(f"ep{p}")
                        for a in range(2):
                            nc.tensor.matmul(
                                pb[0:64, a * QS:(a + 1) * QS],
                                ones_bf[0:1, 0:64],
                                t["recipb"][0:1, a * QS:(a + 1) * QS],
                                start=True, stop=True)

                    def s_bcast_cp():
                        nc.vector.tensor_copy(t["bcast"][:],
                                              t["pb"][0:64, 0:2 * QS])

                    def s_mul():
                        nc.vector.tensor_mul(otn[p][0:64, :], t["pca"][:],
                                             t["bcast"][:, 0:QS])
                        nc.vector.tensor_mul(t["tmpb"][:], t["pcb"][:],
                                             t["bcast"][:, QS:2 * QS])

                    def s_shift():
                        ps2 = t["ps2"] = proj(f"sh{p}")
                        nc.tensor.matmul(ps2[64:128, 0:QS], id_sb[:],
                                         t["tmpb"][:], start=True, stop=True,
                                         tile_position=(0, 64))

                    def s_ocp():
                        nc.vector.tensor_copy(otn[p][64:128, :],
                                              t["ps2"][64:128, 0:QS])

                    return [s_sums, s_ln, s_recip, s_bcast_mm, s_bcast_cp,
                            s_mul, s_shift, s_ocp]

                prev_ep = {"p": None, "pots": None}

                def emit_attn(p, fillers):
                    """fillers: dict chunk_idx -> list of thunks emitted after
                    that chunk's exp. OT emission is deferred (schedule below)
                    so single-buffered pots never block the in-order PE queue:
                    the pot WAR (prev pair's s_sums copies) resolves by chunk
                    ~1.5 and the first OT is emitted at chunk 4."""
                    pots = [pot_pool.tile([65, QS], _f32, tag=f"pot{a}",
                                          name=f"pot{p}_{a}") for a in range(2)]
                    ats_q = []
                    emitted = {"n": 0}

                    def emit_ot():
                        c = emitted["n"]
                        emitted["n"] += 1
                        at = ats_q.pop(0)
                        for a in range(2):
                            nc.tensor.matmul(
                                pots[a][:],
                                v_sb[:, p, c, 65 * a:65 * a + 65],
                                at[:, a * QS:(a + 1) * QS],
                                start=(c == 0), stop=(c == NKC - 1))

                    # chunk -> number of OTs to emit after that chunk's exp
                    ot_sched = {4: 1, 5: 2, 6: 2, 7: 2, 8: 1}

                    for c in range(NKC):
                        pss = slot(f"pss{p}_{c}")
                        for a in range(2):
                            r = slice(64 * a, 64 * a + 64)
                            nc.tensor.matmul(
                                pss[:, a * QS:(a + 1) * QS],
                                kt_sb[r, p, c * 128:(c + 1) * 128],
                                qt_sb[r, p, :],
                                start=True, stop=True)
                        at = attn_pool.tile([128, 2 * QS], _bf, tag="at")
                        nc.scalar.activation(at[:], pss[:], Exp, scale=SCALE)
                        ats_q.append(at)
                        for thunk in fillers.get(c, ()):
                            thunk()
                        if c >= 9:
                            emit_ot()
                        else:
                            for _ in range(ot_sched.get(c, 0)):
                                emit_ot()
                    prev_ep["p"], prev_ep["pots"] = p, pots
                    # OT(15) deferred to the next pair's chunk 0 so the next
                    # scores/exp start before it in PE order
                    return emit_ot

                # software pipeline: proj(0)+qt(0) upfront (during DMA);
                # proj(p+1) and the epilogue(p-1) interleave into attn(p).
                emit_kt(0, 0)
                emit_qt(0)
                pending_ot = None
                for p in range(NPAIR):
                    fillers = {}

                    def put(c, thunk):
                        fillers.setdefault(c, []).append(thunk)

                    if pending_ot is not None:
                        put(0, pending_ot)
                    # own-pair V projection: cast lands ~c2.3/c4.3, first OT
                    # (deferred to c4) reads v chunks 0.. just in time
                    put(1, lambda p=p: emit_v(p, 0))
                    put(3, lambda p=p: emit_v(p, 1))
                    if p == 0:
                        put(5, lambda: emit_kt(0, 1))
                        put(7, lambda: emit_qt(1))
                        put(9, lambda: emit_kt(1, 0))
                        put(11, lambda: emit_kt(1, 1))
                    if prev_ep["pots"] is not None:
                        stages = emit_epilogue_stages(prev_ep["p"],
                                                      prev_ep["pots"])
                        # sums@0 (releases pots), Ln@1/Exp@2 slot into the
                        # ACT FIFO with inputs already ready, then the
                        # bcast/mul/shift tail through the proj region
                        for c, s in zip((0, 1, 2, 5, 6, 7, 8, 9), stages):
                            put(c, s)
                    if p >= 1 and p + 1 < NPAIR:
                        q = p + 1
                        put(11, lambda q=q: emit_qt(q))
                        put(13, lambda q=q: emit_kt(q, 0))
                        put(15, lambda q=q: emit_kt(q, 1))
                    pending_ot = emit_attn(p, fillers)
                pending_ot()

                groups = [(j, dh) for j in range(QS // 128) for dh in range(2)]
                pys = {}

                def y_prefix(g, npre=NPAIR - 1):
                    j, dh = g
                    dsl = slice(dh * 512, (dh + 1) * 512)
                    py = slot(f"y{j}_{dh}")
                    pys[g] = py
                    for k in range(npre):
                        nc.tensor.matmul(
                            py[:, 0:512],
                            otn[k][:, j * 128:(j + 1) * 128],
                            wo_sb[:, k, dsl],
                            start=(k == 0), stop=False)

                def y_finish(g, kfrom=NPAIR - 1):
                    j, dh = g
                    dsl = slice(dh * 512, (dh + 1) * 512)
                    py = pys[g]
                    for k in range(kfrom, NPAIR):
                        nc.tensor.matmul(
                            py[:, 0:512],
                            otn[k][:, j * 128:(j + 1) * 128],
                            wo_sb[:, k, dsl],
                            start=False, stop=False)
                    nc.tensor.matmul(py[:, 0:512], ones_bf[0:1, :],
                                     bo_sb[0:1, dsl],
                                     start=False, stop=True)
                    ysb = y_pool.tile([128, 512], _bf, tag="ysb")
                    nc.vector.tensor_copy(ysb[:], py[:, 0:512])
                    nc.sync.dma_start(
                        out_d[j * 128:(j + 1) * 128, dsl], ysb[:])

                # tail: the last pair's epilogue interleaves with group-0/1
                # prefixes (covering the Pool NR latency) and the y groups
                # pipeline 2-deep through the slot pool so the PE never
                # idles long enough to re-throttle.
                stages = emit_epilogue_stages(prev_ep["p"], prev_ep["pots"])
                s_sums, s_ln, s_recip, s_bcast_mm, s_bcast_cp, s_mul, \
                    s_shift, s_ocp = stages
                s_sums(); s_ln(); s_recip()
                y_prefix(groups[0])
                y_prefix(groups[1])
                s_bcast_mm(); s_bcast_cp(); s_mul(); s_shift(); s_ocp()
                y_finish(groups[0])
                for gi in range(2, len(groups)):
                    y_prefix(groups[gi])
                    y_finish(groups[gi - 1])
                y_finish(groups[-1])

    _split_excess_waits(nc, 1)
    return nc


def _blockdiag_pack(w):
    """[H, HD, HD] -> [128, NPAIR*128] blockdiagonal per pair, k-major."""
    out = np.zeros((128, NPAIR * 128), np.float32)
    for p in range(NPAIR):
        out[0:64, p * 128 + 0:p * 128 + 64] = w[2 * p]
        out[64:128, p * 128 + 64:p * 128 + 128] = w[2 * p + 1]
    return out.astype(BF16)


def _bias_pack(b):
    """[H, HD] -> [128, NPAIR] (pair bias along partitions)."""
    out = np.zeros((128, NPAIR), np.float32)
    for p in range(NPAIR):
        out[0:64, p] = b[2 * p]
        out[64:128, p] = b[2 * p + 1]
    return out


def prepare_inputs(X, Wq, bq, Wk, bk, Wv, bv, Wo, bo):
    """Host-side shard + pack. Returns in_maps (one dict per core).

    bk is accepted but unused: a per-query constant added to every key's
    score cancels exactly in softmax."""
    X = np.asarray(X, np.float32)
    Wo = np.asarray(Wo, np.float32)
    # fold the V bias through the output projection: bo2 = bv @ Wo + bo
    bo2 = (np.asarray(bv, np.float32).reshape(-1) @ Wo
           + np.asarray(bo, np.float32))
    common = {
        "wk": _blockdiag_pack(np.asarray(Wk, np.float32)),
        "wq": _blockdiag_pack(np.asarray(Wq, np.float32)),
        "wv": _blockdiag_pack(np.asarray(Wv, np.float32)),
        "bq": _bias_pack(np.asarray(bq, np.float32)),
        "wo": np.ascontiguousarray(
            Wo.reshape(8, 128, D).transpose(1, 0, 2)
        ).astype(BF16),
        "bo": bo2.reshape(1, D).astype(BF16),
        "ident": np.eye(64, dtype=np.float32).astype(BF16),
    }
    xts = []
    for b in range(B):
        xt = np.ascontiguousarray(X[b].T)                   # [D, S]
        xts.append(np.ascontiguousarray(
            xt.reshape(8, 128, S).transpose(1, 0, 2)).astype(BF16))
    in_maps = []
    for c in range(NCORES):
        b = c // (NCORES // B)
        q0 = (c % (NCORES // B)) * QS
        m = dict(common)
        m["xt"] = xts[b]
        m["xtq"] = np.ascontiguousarray(xts[b][:, :, q0:q0 + QS])
        in_maps.append(m)
    return in_maps


_NC_CACHE = None


def _get_nc():
    global _NC_CACHE
    if _NC_CACHE is None:
        _NC_CACHE = build_nc()
    return _NC_CACHE


def kernel(X, Wq, bq, Wk, bk, Wv, bv, Wo, bo):
    nc = _get_nc()
    in_maps = prepare_inputs(X, Wq, bq, Wk, bk, Wv, bv, Wo, bo)
    res = run_bass_kernel_spmd(nc, in_maps, core_ids=list(range(NCORES)))
    out = np.empty((B, S, D), np.float32)
    for c in range(NCORES):
        b = c // (NCORES // B)
        q0 = (c % (NCORES // B)) * QS
        out[b, q0:q0 + QS, :] = np.asarray(res.results[c]["out"],
                                           dtype=np.float32)
    return out


# revision 22
# speedup vs baseline: 4.2925x; 1.0111x over previous
"""Bass/Trainium2 kernel for nn_BeMultiHeadAttention (B=2, S=2048, D=1024, H=16, HD=64).

Sharding: data-parallel over tokens. 8 cores; core c handles batch b=c//4 and
query slice q0=(c%4)*512 .. +512. Each core computes K/V projections for its
full batch (2048 keys), Q projection for its 512 queries, transposed-scores
flash attention (no max subtraction needed: |score/8| <~ 2), and the output
projection for its 512 tokens. No collectives; the host concatenates shards.

v2 layout notes:
 - The K-projection bias is key-independent inside softmax (adds bk.q to every
   key's score), so it cancels exactly and is dropped entirely.
 - ACT runs PURE exp (128 x [128,1024] chunks). The softmax reciprocal moved
   to DVE reciprocal_approx_fast; the per-query broadcast stays a rank-1 PE
   matmul into the shared proj PSUM region.
 - PSUM (8 banks): scores slots 2x[128,1024] (4) + potA/potB [65,512] (2,
   single-buffered) + proj/epilogue region [128,1024] (2). The scores slots
   are claimed ONLY by scores chunks -> pure 2-deep alternation, so
   scores(c+2) waits exp(c) which ended a full chunk earlier: ACT never gaps
   on the slot rotation (the baseline lost ~535ns on most chunks to fillers
   flipping the claim parity).
 - Single-buffered pots work because the epilogue copies pots to SBUF (f32)
   immediately at pair start, releasing the banks by chunk ~1.5; the next
   pair's OT emission is deferred to chunks 4..8 (the 6-deep at pool absorbs
   the lag) so the in-order PE queue never blocks on the pot WAR.
 - Projections for pair p+1 and the epilogue of pair p-1 serialize through
   the proj region (claims >= 2 chunks apart so each claim's WAR on the
   previous claim's DVE reader is already satisfied at issue).
 - DMA: consolidated big-line transfers (xt per pair = 4KB/partition lines),
   critical pair-0 set first in first-use order.
 - Output bias via a K=1 matmul (ones row x bo2) appended to the y
   accumulation; bo2 = bv @ Wo + bo folds the V bias.
"""

import numpy as np
import ml_dtypes

import concourse.bass as bass
import concourse.tile as tile
import concourse.mybir as mybir
from concourse.bass_utils import run_bass_kernel_spmd


BF16 = ml_dtypes.bfloat16

B, S, D, H, HD = 2, 2048, 1024, 16, 64
NCORES = 8
QS = S * B // NCORES          # 512 queries per core
NPAIR = H // 2                # 8 head pairs
NKC = S // 128                # 16 key chunks
SCALE = 1.0 / np.sqrt(HD)     # 0.125

_bf = mybir.dt.bfloat16
_f32 = mybir.dt.float32


def _split_excess_waits(nc, max_waits=1):
    """This container's walrus only accepts one sync-wait per instruction;
    split extras onto preceding NoOps on the same engine."""
    for fn in nc.m.functions:
        for bb in fn.blocks:
            new_insts = []
            for inst in bb.instructions:
                si = inst.sync_info
                if si is not None and si.on_wait and len(si.on_wait) > max_waits:
                    waits = list(si.on_wait)
                    extra, keep = waits[:-max_waits], waits[-max_waits:]
                    while extra:
                        chunk, extra = extra[:max_waits], extra[max_waits:]
                        new_insts.append(mybir.InstNoOp(
                            name=nc.get_next_instruction_name(),
                            engine=inst.engine,
                            sync_info=mybir.SyncInfo(on_wait=chunk, on_update=[]),
                            bass_nofuse=True))
                    inst.sync_info = mybir.SyncInfo(
                        on_wait=keep, on_update=list(si.on_update))
                new_insts.append(inst)
            bb.instructions = new_insts


def build_nc():
    nc = bass.Bass("TRN2", target_bir_lowering=False, debug=False)

    xt_in = nc.declare_dram_parameter("xt", [128, 8, S], _bf, isOutput=False)
    xtq_in = nc.declare_dram_parameter("xtq", [128, 8, QS], _bf, isOutput=False)
    wk_in = nc.declare_dram_parameter("wk", [128, NPAIR * 128], _bf, isOutput=False)
    wq_in = nc.declare_dram_parameter("wq", [128, NPAIR * 128], _bf, isOutput=False)
    wv_in = nc.declare_dram_parameter("wv", [128, NPAIR * 128], _bf, isOutput=False)
    bq_in = nc.declare_dram_parameter("bq", [128, NPAIR], _f32, isOutput=False)
    wo_in = nc.declare_dram_parameter("wo", [128, 8, D], _bf, isOutput=False)
    bo_in = nc.declare_dram_parameter("bo", [1, D], _bf, isOutput=False)
    id_in = nc.declare_dram_parameter("ident", [64, 64], _bf, isOutput=False)
    # bf16 output halves the output-DMA drain at the tail; the host upcasts.
    out_d = nc.declare_dram_parameter("out", [QS, D], _bf, isOutput=True)

    Exp = mybir.ActivationFunctionType.Exp

    with tile.TileContext(nc) as tc:
        with (
            tc.tile_pool(name="singles", bufs=1) as singles,
            tc.tile_pool(name="attn", bufs=6) as attn_pool,
            tc.tile_pool(name="ep", bufs=2) as ep_pool,
            tc.tile_pool(name="ysb", bufs=2) as y_pool,
        ):
            ones_bf = singles.tile([1, 128], _bf)
            nc.vector.memset(ones_bf[:], 1.0)
            warm_rhs = singles.tile([1, 512], _bf)
            nc.vector.memset(warm_rhs[:], 1.0)

            wk_sb = singles.tile([128, NPAIR * 128], _bf)
            wq_sb = singles.tile([128, NPAIR * 128], _bf)
            wv_sb = singles.tile([128, NPAIR * 128], _bf)
            bq_sb = singles.tile([128, NPAIR], _f32)
            bo_sb = singles.tile([1, D], _bf)
            id_sb = singles.tile([64, 64], _bf)
            xtq_sb = singles.tile([128, 8, QS], _bf)
            xt_sb = singles.tile([128, 8, S], _bf)
            wo_sb = singles.tile([128, 8, D], _bf)

            # --- DMA emission: pair-0 critical set first, in first-use order
            # (kt g0 needs wk + xt cols 0:1024; qt needs wq + bq + xtq p0).
            # The kt-path inputs go on the scalar HWDGE ring so their issue
            # overlaps the qt-path issues on the sync ring. Full-tensor
            # weight DMAs get 2KB/partition lines; per-pair xt gets 4KB.
            nc.scalar.dma_start(wk_sb[:], wk_in[:])
            nc.sync.dma_start(wq_sb[:], wq_in[:])
            nc.scalar.dma_start(xt_sb[:, 0, 0:1024], xt_in[:, 0, 0:1024])
            nc.sync.dma_start(xtq_sb[:, 0, :], xtq_in[:, 0, :])
            nc.sync.dma_start(bq_sb[:], bq_in[:])
            nc.sync.dma_start(id_sb[:], id_in[:])
            nc.sync.dma_start(xt_sb[:, 0, 1024:2048], xt_in[:, 0, 1024:2048])
            nc.sync.dma_start(wv_sb[:], wv_in[:])
            for p in range(1, NPAIR):
                nc.sync.dma_start(xt_sb[:, p, :], xt_in[:, p, :])
                nc.sync.dma_start(xtq_sb[:, p, :], xtq_in[:, p, :])
            nc.sync.dma_start(bo_sb[:], bo_in[:])
            nc.sync.dma_start(wo_sb[:], wo_in[:])

            kt_sb = singles.tile([128, NPAIR, S], _bf)
            qt_sb = singles.tile([128, NPAIR, QS], _bf)
            # V layout per (pair, keychunk): [V_A(64) | ones | V_B(64) | ones]
            v_sb = singles.tile([128, NPAIR, NKC, 130], _bf)
            # only the two ones-columns need the memset; 130 = 2*65 so the
            # ones-columns form a uniform stride-65 pattern (3-dim AP)
            nc.vector.memset(
                v_sb.rearrange("p a c (h e) -> p (a c h) e", e=65)[:, :, 64:65],
                1.0)

            otn = [singles.tile([128, QS], _bf, name=f"otn{p}") for p in range(NPAIR)]

            # epilogue SBUF scratch (per pair, double-buffered)
            def ep_tiles():
                return dict(
                    sums=ep_pool.tile([1, 2 * QS], _f32, tag="sums",
                                      name="sums"),
                    lnrow=ep_pool.tile([1, 2 * QS], _f32, tag="lnrow",
                                       name="lnrow"),
                    recipb=ep_pool.tile([1, 2 * QS], _bf, tag="recipb",
                                        name="recipb"),
                    bcast=ep_pool.tile([64, 2 * QS], _f32, tag="bcast",
                                       name="bcast"),
                    pca=ep_pool.tile([64, QS], _f32, tag="pca", name="pca"),
                    pcb=ep_pool.tile([64, QS], _f32, tag="pcb", name="pcb"),
                    tmpb=ep_pool.tile([64, QS], _bf, tag="tmpb", name="tmpb"),
                )

            # PSUM: scores 2x[128,1024] (4 banks, scores-only claims) +
            # potA/potB [65,512] single-buffered (2 banks) + proj [128,1024]
            # (2 banks, serialized claims for projections/epilogue/tail-y).
            with (
                tc.tile_pool(name="pslot", bufs=2, space="PSUM") as slot_pool,
                tc.tile_pool(name="ppot", bufs=1, space="PSUM") as pot_pool,
                tc.tile_pool(name="pproj", bufs=1, space="PSUM") as proj_pool,
            ):
                def slot(nm):
                    return slot_pool.tile([128, 1024], _f32, tag="slot", name=nm)

                def proj(nm):
                    return proj_pool.tile([128, 1024], _f32, tag="proj", name=nm)

                # PE warm-up: dummy matmuls (dep only on memsets) ramp the HAM
                # clock gate while the pair-0 DMAs land.
                wps = slot("warm")
                for i in range(12):
                    nc.tensor.matmul(wps[:, 0:512], ones_bf[:], warm_rhs[:],
                                     start=True, stop=True)

                # ---- projection units (each claims the proj region once) ----
                def emit_kt(p, g):
                    """kt for pair p, keys [g*1024, (g+1)*1024): 2 MMs + cast."""
                    ws = slice(p * 128, (p + 1) * 128)
                    ps = proj(f"kt{p}_{g}")
                    for i in range(2):
                        t0 = g * 1024 + i * 512
                        nc.tensor.matmul(
                            ps[:, i * 512:(i + 1) * 512],
                            wk_sb[:, ws],
                            xt_sb[:, p, t0:t0 + 512],
                            start=True, stop=True)
                    nc.vector.tensor_copy(
                        kt_sb[:, p, g * 1024:(g + 1) * 1024], ps[:])

                def emit_qt(p, region=None):
                    psq = proj(f"qt{p}") if region is None else region
                    ws = slice(p * 128, (p + 1) * 128)
                    nc.tensor.matmul(psq[:, 0:QS], wq_sb[:, ws], xtq_sb[:, p, :],
                                     start=True, stop=True)
                    nc.vector.tensor_scalar_add(
                        qt_sb[:, p, :], psq[:, 0:QS], bq_sb[:, p:p + 1])

                def emit_v(p, g):
                    """V for pair p, key chunks [g*8, g*8+8): 8 MMs + cast."""
                    ws = slice(p * 128, (p + 1) * 128)
                    psv = proj(f"v{p}_{g}")
                    psv8 = psv.rearrange("p (c e) -> p c e", e=128)
                    for i in range(8):
                        c = g * 8 + i
                        nc.tensor.matmul(
                            psv8[:, i, :],
                            xt_sb[:, p, c * 128:(c + 1) * 128],
                            wv_sb[:, ws],
                            start=True, stop=True)
                    dst = v_sb[:, p, g * 8:(g + 1) * 8, :].rearrange(
                        "p c (h e) -> p c h e", e=65)[:, :, :, 0:64]
                    src = psv[:, 0:1024].rearrange(
                        "p (c h e) -> p c h e", h=2, e=64)
                    nc.vector.tensor_copy(dst, src)

                # ---- epilogue for pair p as staged thunks ----
                def emit_epilogue_stages(p, pots, reuse_pb=False):
                    """reuse_pb: the tail variant claims the proj region once
                    (s_bcast_mm) and the shift writes its unused partitions
                    64-127 — later tail claims of the proj region must not
                    create a WAR cycle through the y-group casts."""
                    t = ep_tiles()

                    def s_sums():
                        # softmax sums (f32) + unnormalized outputs to SBUF;
                        # pots are fully released after these reads.
                        for a, pc in ((0, t["pca"]), (1, t["pcb"])):
                            nc.vector.tensor_copy(
                                t["sums"][0:1, a * QS:(a + 1) * QS],
                                pots[a][64:65, :])
                            nc.vector.tensor_copy(pc[:], pots[a][0:64, :])

                    def s_ln():
                        # 1/Z as exp(-ln Z): both anchors live in the
                        # natural_log_exp_and_others table set (no reload),
                        # and [1,1024]-row transcendentals are cheapest on
                        # ACT (~1.15us each; DVE/Pool lack a usable divide).
                        nc.scalar.activation(t["lnrow"][0:1, :],
                                             t["sums"][0:1, :],
                                             mybir.ActivationFunctionType.Ln)

                    def s_recip():
                        nc.scalar.activation(t["recipb"][0:1, :],
                                             t["lnrow"][0:1, :],
                                             Exp, scale=-1.0)

                    def s_bcast_mm():
                        pb = t["pb"] = proj(f"ep{p}")
                        for a in range(2):
                            nc.tensor.matmul(
                                pb[0:64, a * QS:(a + 1) * QS],
                                ones_bf[0:1, 0:64],
                                t["recipb"][0:1, a * QS:(a + 1) * QS],
                                start=True, stop=True)

                    def s_bcast_cp():
                        nc.vector.tensor_copy(t["bcast"][:],
                                              t["pb"][0:64, 0:2 * QS])

                    def s_mul():
                        nc.vector.tensor_mul(otn[p][0:64, :], t["pca"][:],
                                             t["bcast"][:, 0:QS])
                        nc.vector.tensor_mul(t["tmpb"][:], t["pcb"][:],
                                             t["bcast"][:, QS:2 * QS])

                    def s_shift():
                        if reuse_pb:
                            ps2 = t["ps2"] = t["pb"]
                        else:
                            ps2 = t["ps2"] = proj(f"sh{p}")
                        nc.tensor.matmul(ps2[64:128, 0:QS], id_sb[:],
                                         t["tmpb"][:], start=True, stop=True,
                                         tile_position=(0, 64))

                    def s_ocp():
                        nc.vector.tensor_copy(otn[p][64:128, :],
                                              t["ps2"][64:128, 0:QS])

                    return [s_sums, s_ln, s_recip, s_bcast_mm, s_bcast_cp,
                            s_mul, s_shift, s_ocp]

                prev_ep = {"p": None, "pots": None}

                def emit_attn(p, fillers):
                    """fillers: dict chunk_idx -> list of thunks emitted after
                    that chunk's exp. OT emission is deferred (schedule below)
                    so single-buffered pots never block the in-order PE queue:
                    the pot WAR (prev pair's s_sums copies) resolves by chunk
                    ~1.5 and the first OT is emitted at chunk 4."""
                    pots = [pot_pool.tile([65, QS], _f32, tag=f"pot{a}",
                                          name=f"pot{p}_{a}") for a in range(2)]
                    ats_q = []
                    emitted = {"n": 0}

                    def emit_ot():
                        c = emitted["n"]
                        emitted["n"] += 1
                        at = ats_q.pop(0)
                        for a in range(2):
                            nc.tensor.matmul(
                                pots[a][:],
                                v_sb[:, p, c, 65 * a:65 * a + 65],
                                at[:, a * QS:(a + 1) * QS],
                                start=(c == 0), stop=(c == NKC - 1))

                    # chunk -> number of OTs to emit after that chunk's exp
                    ot_sched = {4: 1, 5: 2, 6: 2, 7: 2, 8: 1}

                    for c in range(NKC):
                        pss = slot(f"pss{p}_{c}")
                        for a in range(2):
                            r = slice(64 * a, 64 * a + 64)
                            nc.tensor.matmul(
                                pss[:, a * QS:(a + 1) * QS],
                                kt_sb[r, p, c * 128:(c + 1) * 128],
                                qt_sb[r, p, :],
                                start=True, stop=True)
                        at = attn_pool.tile([128, 2 * QS], _bf, tag="at")
                        nc.scalar.activation(at[:], pss[:], Exp, scale=SCALE)
                        ats_q.append(at)
                        for thunk in fillers.get(c, ()):
                            thunk()
                        if c >= 9:
                            emit_ot()
                        else:
                            for _ in range(ot_sched.get(c, 0)):
                                emit_ot()
                    prev_ep["p"], prev_ep["pots"] = p, pots
                    # OT(15) deferred to the next pair's chunk 0 so the next
                    # scores/exp start before it in PE order
                    return emit_ot

                # software pipeline: kt(0)g0 (proj region) and qt(0) (slot
                # region, so the two chains don't serialize through the
                # single proj bank while DMAs land) run upfront; proj(p+1)
                # and the epilogue(p-1) interleave into attn(p). The extra
                # slot claim pairs with the warmup claim, keeping the
                # scores-slot parity intact.
                emit_qt(0, region=slot("qt0s"))
                emit_kt(0, 0)
                pending_ot = None
                for p in range(NPAIR):
                    fillers = {}

                    def put(c, thunk):
                        fillers.setdefault(c, []).append(thunk)

                    if pending_ot is not None:
                        put(0, pending_ot)
                    # own-pair V projection: cast lands ~c2.3/c4.3, first OT
                    # (deferred to c4) reads v chunks 0.. just in time
                    put(1, lambda p=p: emit_v(p, 0))
                    put(3, lambda p=p: emit_v(p, 1))
                    if p == 0:
                        put(5, lambda: emit_kt(0, 1))
                        put(7, lambda: emit_qt(1))
                        put(9, lambda: emit_kt(1, 0))
                        put(11, lambda: emit_kt(1, 1))
                    if prev_ep["pots"] is not None:
                        stages = emit_epilogue_stages(prev_ep["p"],
                                                      prev_ep["pots"])
                        # sums@0 (releases pots), Ln@1/Exp@2 slot into the
                        # ACT FIFO with inputs already ready, then the
                        # bcast/mul/shift tail through the proj region
                        for c, s in zip((0, 1, 2, 5, 6, 7, 8, 9), stages):
                            put(c, s)
                    if p >= 1 and p + 1 < NPAIR:
                        # kt g1 lands at c14 so the pair boundary carries
                        # only OT(14): scores(p+1,0) reaches the PE sooner
                        # and exp never waits at the transition
                        q = p + 1
                        put(10, lambda q=q: emit_qt(q))
                        put(12, lambda q=q: emit_kt(q, 0))
                        put(14, lambda q=q: emit_kt(q, 1))
                    pending_ot = emit_attn(p, fillers)
                pending_ot()

                groups = [(j, dh) for j in range(QS // 128) for dh in range(2)]
                pys = {}

                def y_prefix(g, py, npre=NPAIR - 1):
                    j, dh = g
                    dsl = slice(dh * 512, (dh + 1) * 512)
                    pys[g] = py
                    for k in range(npre):
                        nc.tensor.matmul(
                            py[:, 0:512],
                            otn[k][:, j * 128:(j + 1) * 128],
                            wo_sb[:, k, dsl],
                            start=(k == 0), stop=False)

                def y_finish(g):
                    j, dh = g
                    dsl = slice(dh * 512, (dh + 1) * 512)
                    py = pys[g]
                    nc.tensor.matmul(
                        py[:, 0:512],
                        otn[NPAIR - 1][:, j * 128:(j + 1) * 128],
                        wo_sb[:, NPAIR - 1, dsl],
                        start=False, stop=False)
                    nc.tensor.matmul(py[:, 0:512], ones_bf[0:1, :],
                                     bo_sb[0:1, dsl],
                                     start=False, stop=True)
                    ysb = y_pool.tile([128, 512], _bf, tag="ysb")
                    nc.vector.tensor_copy(ysb[:], py[:, 0:512])
                    nc.sync.dma_start(
                        out_d[j * 128:(j + 1) * 128, dsl], ysb[:])

                # tail: every PSUM resource is free once the last exp has
                # read its slot, so ALL 8 y groups accumulate concurrently
                # (2 slot bufs + the proj region give 6 bank-halves; the two
                # pot banks are reclaimed as [128,512] tiles for the rest).
                # 56 back-to-back prefix matmuls hide the whole epilogue
                # chain and keep the PE clock warm through the transition.
                stages = emit_epilogue_stages(prev_ep["p"], prev_ep["pots"],
                                              reuse_pb=True)
                s_sums, s_ln, s_recip, s_bcast_mm, s_bcast_cp, s_mul, \
                    s_shift, s_ocp = stages
                s_sums(); s_ln(); s_recip()
                yA = slot("yA")
                yB = slot("yB")
                yD = pot_pool.tile([128, QS], _f32, tag="pot0", name="yD")
                yE = pot_pool.tile([128, QS], _f32, tag="pot1", name="yE")
                y_prefix(groups[0], yA[:, 0:512])
                y_prefix(groups[1], yA[:, 512:1024])
                s_bcast_mm()          # claims the proj region (shift reuses)
                y_prefix(groups[2], yB[:, 0:512])
                y_prefix(groups[3], yB[:, 512:1024])
                s_bcast_cp(); s_mul(); s_shift(); s_ocp()
                y_prefix(groups[4], yD[:, 0:512])
                y_prefix(groups[5], yE[:, 0:512])
                # the proj region frees once ocp has read the shift output;
                # claim it last for the final two groups
                yC = proj_pool.tile([128, 1024], _f32, tag="proj", name="yC")
                y_prefix(groups[6], yC[:, 0:512])
                y_prefix(groups[7], yC[:, 512:1024])
                for g in groups:
                    y_finish(g)

    _split_excess_waits(nc, 1)
    return nc


def _blockdiag_pack(w):
    """[H, HD, HD] -> [128, NPAIR*128] blockdiagonal per pair, k-major."""
    out = np.zeros((128, NPAIR * 128), np.float32)
    for p in range(NPAIR):
        out[0:64, p * 128 + 0:p * 128 + 64] = w[2 * p]
        out[64:128, p * 128 + 64:p * 128 + 128] = w[2 * p + 1]
    return out.astype(BF16)


def _bias_pack(b):
    """[H, HD] -> [128, NPAIR] (pair bias along partitions)."""
    out = np.zeros((128, NPAIR), np.float32)
    for p in range(NPAIR):
        out[0:64, p] = b[2 * p]
        out[64:128, p] = b[2 * p + 1]
    return out


def prepare_inputs(X, Wq, bq, Wk, bk, Wv, bv, Wo, bo):
    """Host-side shard + pack. Returns in_maps (one dict per core).

    bk is accepted but unused: a per-query constant added to every key's
    score cancels exactly in softmax."""
    X = np.asarray(X, np.float32)
    Wo = np.asarray(Wo, np.float32)
    # fold the V bias through the output projection: bo2 = bv @ Wo + bo
    bo2 = (np.asarray(bv, np.float32).reshape(-1) @ Wo
           + np.asarray(bo, np.float32))
    common = {
        "wk": _blockdiag_pack(np.asarray(Wk, np.float32)),
        "wq": _blockdiag_pack(np.asarray(Wq, np.float32)),
        "wv": _blockdiag_pack(np.asarray(Wv, np.float32)),
        "bq": _bias_pack(np.asarray(bq, np.float32)),
        "wo": np.ascontiguousarray(
            Wo.reshape(8, 128, D).transpose(1, 0, 2)
        ).astype(BF16),
        "bo": bo2.reshape(1, D).astype(BF16),
        "ident": np.eye(64, dtype=np.float32).astype(BF16),
    }
    xts = []
    for b in range(B):
        xt = np.ascontiguousarray(X[b].T)                   # [D, S]
        xts.append(np.ascontiguousarray(
            xt.reshape(8, 128, S).transpose(1, 0, 2)).astype(BF16))
    in_maps = []
    for c in range(NCORES):
        b = c // (NCORES // B)
        q0 = (c % (NCORES // B)) * QS
        m = dict(common)
        m["xt"] = xts[b]
        m["xtq"] = np.ascontiguousarray(xts[b][:, :, q0:q0 + QS])
        in_maps.append(m)
    return in_maps


_NC_CACHE = None


def _get_nc():
    global _NC_CACHE
    if _NC_CACHE is None:
        _NC_CACHE = build_nc()
    return _NC_CACHE


def kernel(X, Wq, bq, Wk, bk, Wv, bv, Wo, bo):
    nc = _get_nc()
    in_maps = prepare_inputs(X, Wq, bq, Wk, bk, Wv, bv, Wo, bo)
    res = run_bass_kernel_spmd(nc, in_maps, core_ids=list(range(NCORES)))
    out = np.empty((B, S, D), np.float32)
    for c in range(NCORES):
        b = c // (NCORES // B)
        q0 = (c % (NCORES // B)) * QS
        out[b, q0:q0 + QS, :] = np.asarray(res.results[c]["out"],
                                           dtype=np.float32)
    return out


# revision 29
# speedup vs baseline: 4.4834x; 1.0445x over previous
"""Bass/Trainium2 kernel for nn_BeMultiHeadAttention (B=2, S=2048, D=1024, H=16, HD=64).

Sharding: data-parallel over tokens. 8 cores; core c handles batch b=c//4 and
query slice q0=(c%4)*512 .. +512. Each core computes K/V projections for its
full batch (2048 keys), Q projection for its 512 queries, transposed-scores
flash attention (no max subtraction needed: |score/8| <~ 2), and the output
projection for its 512 tokens. No collectives; the host concatenates shards.

v2 layout notes:
 - The K-projection bias is key-independent inside softmax (adds bk.q to every
   key's score), so it cancels exactly and is dropped entirely.
 - ACT runs PURE exp (128 x [128,1024] chunks). The softmax reciprocal moved
   to DVE reciprocal_approx_fast; the per-query broadcast stays a rank-1 PE
   matmul into the shared proj PSUM region.
 - PSUM (8 banks): scores slots 2x[128,1024] (4) + potA/potB [65,512] (2,
   single-buffered) + proj/epilogue region [128,1024] (2). The scores slots
   are claimed ONLY by scores chunks -> pure 2-deep alternation, so
   scores(c+2) waits exp(c) which ended a full chunk earlier: ACT never gaps
   on the slot rotation (the baseline lost ~535ns on most chunks to fillers
   flipping the claim parity).
 - Single-buffered pots work because the epilogue copies pots to SBUF (f32)
   immediately at pair start, releasing the banks by chunk ~1.5; the next
   pair's OT emission is deferred to chunks 4..8 (the 6-deep at pool absorbs
   the lag) so the in-order PE queue never blocks on the pot WAR.
 - Projections for pair p+1 and the epilogue of pair p-1 serialize through
   the proj region (claims >= 2 chunks apart so each claim's WAR on the
   previous claim's DVE reader is already satisfied at issue).
 - DMA: consolidated big-line transfers (xt per pair = 4KB/partition lines),
   critical pair-0 set first in first-use order.
 - Output bias via a K=1 matmul (ones row x bo2) appended to the y
   accumulation; bo2 = bv @ Wo + bo folds the V bias.
"""

import numpy as np
import ml_dtypes

import concourse.bass as bass
import concourse.tile as tile
import concourse.mybir as mybir
from concourse.bass_utils import run_bass_kernel_spmd


BF16 = ml_dtypes.bfloat16

B, S, D, H, HD = 2, 2048, 1024, 16, 64
NCORES = 8
QS = S * B // NCORES          # 512 queries per core
NPAIR = H // 2                # 8 head pairs
NKC = S // 128                # 16 key chunks
SCALE = 1.0 / np.sqrt(HD)     # 0.125

_bf = mybir.dt.bfloat16
_f32 = mybir.dt.float32


def _split_excess_waits(nc, max_waits=1):
    """This container's walrus only accepts one sync-wait per instruction;
    split extras onto preceding NoOps on the same engine."""
    for fn in nc.m.functions:
        for bb in fn.blocks:
            new_insts = []
            for inst in bb.instructions:
                si = inst.sync_info
                if si is not None and si.on_wait and len(si.on_wait) > max_waits:
                    waits = list(si.on_wait)
                    extra, keep = waits[:-max_waits], waits[-max_waits:]
                    while extra:
                        chunk, extra = extra[:max_waits], extra[max_waits:]
                        new_insts.append(mybir.InstNoOp(
                            name=nc.get_next_instruction_name(),
                            engine=inst.engine,
                            sync_info=mybir.SyncInfo(on_wait=chunk, on_update=[]),
                            bass_nofuse=True))
                    inst.sync_info = mybir.SyncInfo(
                        on_wait=keep, on_update=list(si.on_update))
                new_insts.append(inst)
            bb.instructions = new_insts


def build_nc():
    nc = bass.Bass("TRN2", target_bir_lowering=False, debug=False)

    xt_in = nc.declare_dram_parameter("xt", [128, 8, S], _bf, isOutput=False)
    xtq_in = nc.declare_dram_parameter("xtq", [128, 8, QS], _bf, isOutput=False)
    wk_in = nc.declare_dram_parameter("wk", [128, NPAIR * 128], _bf, isOutput=False)
    wq_in = nc.declare_dram_parameter("wq", [128, NPAIR * 128], _bf, isOutput=False)
    wv_in = nc.declare_dram_parameter("wv", [128, NPAIR * 128], _bf, isOutput=False)
    bq_in = nc.declare_dram_parameter("bq", [128, NPAIR], _f32, isOutput=False)
    wo_in = nc.declare_dram_parameter("wo", [128, 8, D], _bf, isOutput=False)
    bo_in = nc.declare_dram_parameter("bo", [1, D], _bf, isOutput=False)
    id_in = nc.declare_dram_parameter("ident", [64, 64], _bf, isOutput=False)
    # bf16 output halves the output-DMA drain at the tail; the host upcasts.
    out_d = nc.declare_dram_parameter("out", [QS, D], _bf, isOutput=True)

    Exp = mybir.ActivationFunctionType.Exp

    with tile.TileContext(nc) as tc:
        with (
            tc.tile_pool(name="singles", bufs=1) as singles,
            tc.tile_pool(name="attn", bufs=6) as attn_pool,
            tc.tile_pool(name="ep", bufs=2) as ep_pool,
            tc.tile_pool(name="ysb", bufs=4) as y_pool,
        ):
            ones_bf = singles.tile([1, 128], _bf)
            nc.vector.memset(ones_bf[:], 1.0)
            warm_rhs = singles.tile([1, 512], _bf)
            nc.vector.memset(warm_rhs[:], 1.0)

            wk_sb = singles.tile([128, NPAIR * 128], _bf)
            wq_sb = singles.tile([128, NPAIR * 128], _bf)
            wv_sb = singles.tile([128, NPAIR * 128], _bf)
            bq_sb = singles.tile([128, NPAIR], _f32)
            bo_sb = singles.tile([1, D], _bf)
            id_sb = singles.tile([64, 64], _bf)
            xtq_sb = singles.tile([128, 8, QS], _bf)
            xt_sb = singles.tile([128, 8, S], _bf)
            wo_sb = singles.tile([128, 8, D], _bf)

            # --- DMA emission: pair-0 critical set first, in first-use order
            # (kt g0 needs wk + xt cols 0:1024; qt needs wq + bq + xtq p0).
            # The kt-path inputs go on the scalar HWDGE ring so their issue
            # overlaps the qt-path issues on the sync ring. Full-tensor
            # weight DMAs get 2KB/partition lines; per-pair xt gets 4KB.
            nc.scalar.dma_start(wk_sb[:], wk_in[:])
            nc.sync.dma_start(xtq_sb[:, 0, :], xtq_in[:, 0, :])
            nc.scalar.dma_start(xt_sb[:, 0, 0:512], xt_in[:, 0, 0:512])
            nc.sync.dma_start(wq_sb[:], wq_in[:])
            nc.scalar.dma_start(xt_sb[:, 0, 512:1024], xt_in[:, 0, 512:1024])
            nc.sync.dma_start(bq_sb[:], bq_in[:])
            nc.sync.dma_start(wv_sb[:], wv_in[:])
            nc.sync.dma_start(id_sb[:], id_in[:])
            nc.sync.dma_start(xt_sb[:, 0, 1024:2048], xt_in[:, 0, 1024:2048])
            for p in range(1, NPAIR):
                nc.sync.dma_start(xt_sb[:, p, :], xt_in[:, p, :])
                nc.sync.dma_start(xtq_sb[:, p, :], xtq_in[:, p, :])
            nc.sync.dma_start(bo_sb[:], bo_in[:])
            nc.sync.dma_start(wo_sb[:], wo_in[:])

            kt_sb = singles.tile([128, NPAIR, S], _bf)
            qt_sb = singles.tile([128, NPAIR, QS], _bf)
            # V layout per (pair, keychunk): [V_A(64) | ones | V_B(64) | ones]
            v_sb = singles.tile([128, NPAIR, NKC, 130], _bf)
            # only the two ones-columns need the memset; 130 = 2*65 so the
            # ones-columns form a uniform stride-65 pattern (3-dim AP)
            nc.vector.memset(
                v_sb.rearrange("p a c (h e) -> p (a c h) e", e=65)[:, :, 64:65],
                1.0)

            otn = [singles.tile([128, QS], _bf, name=f"otn{p}") for p in range(NPAIR)]

            # epilogue SBUF scratch (per pair, double-buffered)
            def ep_tiles():
                return dict(
                    sums=ep_pool.tile([1, 2 * QS], _f32, tag="sums",
                                      name="sums"),
                    lnrow=ep_pool.tile([1, 2 * QS], _f32, tag="lnrow",
                                       name="lnrow"),
                    recipb=ep_pool.tile([1, 2 * QS], _bf, tag="recipb",
                                        name="recipb"),
                    bcast=ep_pool.tile([64, 2 * QS], _f32, tag="bcast",
                                       name="bcast"),
                    pca=ep_pool.tile([64, QS], _f32, tag="pca", name="pca"),
                    pcb=ep_pool.tile([64, QS], _f32, tag="pcb", name="pcb"),
                    tmpb=ep_pool.tile([64, QS], _bf, tag="tmpb", name="tmpb"),
                )

            # PSUM: scores 2x[128,1024] (4 banks, scores-only claims) +
            # potA/potB [65,512] single-buffered (2 banks) + proj [128,1024]
            # (2 banks, serialized claims for projections/epilogue/tail-y).
            with (
                tc.tile_pool(name="pslot", bufs=2, space="PSUM") as slot_pool,
                tc.tile_pool(name="ppot", bufs=1, space="PSUM") as pot_pool,
                tc.tile_pool(name="pproj", bufs=1, space="PSUM") as proj_pool,
            ):
                def slot(nm):
                    return slot_pool.tile([128, 1024], _f32, tag="slot", name=nm)

                def proj(nm):
                    return proj_pool.tile([128, 1024], _f32, tag="proj", name=nm)

                # PE warm-up: dummy matmuls (dep only on memsets) ramp the HAM
                # clock gate while the pair-0 DMAs land.
                wps = slot("warm")
                for i in range(12):
                    nc.tensor.matmul(wps[:, 0:512], ones_bf[:], warm_rhs[:],
                                     start=True, stop=True)

                # ---- projection units (each claims the proj region once) ----
                def emit_kt(p, g, split=False):
                    """kt for pair p, keys [g*1024, (g+1)*1024): 2 MMs + cast.
                    split: MM/cast interleaved per 512-key half so the first
                    scores chunk unblocks one cast earlier (startup)."""
                    ws = slice(p * 128, (p + 1) * 128)
                    ps = proj(f"kt{p}_{g}")
                    for i in range(2):
                        t0 = g * 1024 + i * 512
                        nc.tensor.matmul(
                            ps[:, i * 512:(i + 1) * 512],
                            wk_sb[:, ws],
                            xt_sb[:, p, t0:t0 + 512],
                            start=True, stop=True)
                        if split:
                            nc.vector.tensor_copy(
                                kt_sb[:, p, t0:t0 + 512],
                                ps[:, i * 512:(i + 1) * 512])
                    if not split:
                        nc.vector.tensor_copy(
                            kt_sb[:, p, g * 1024:(g + 1) * 1024], ps[:])

                def emit_qt(p, region=None):
                    psq = proj(f"qt{p}") if region is None else region
                    ws = slice(p * 128, (p + 1) * 128)
                    nc.tensor.matmul(psq[:, 0:QS], wq_sb[:, ws], xtq_sb[:, p, :],
                                     start=True, stop=True)
                    nc.vector.tensor_scalar_add(
                        qt_sb[:, p, :], psq[:, 0:QS], bq_sb[:, p:p + 1])

                def emit_v(p, g):
                    """V for pair p, key chunks [g*8, g*8+8): 8 MMs + cast."""
                    ws = slice(p * 128, (p + 1) * 128)
                    psv = proj(f"v{p}_{g}")
                    psv8 = psv.rearrange("p (c e) -> p c e", e=128)
                    for i in range(8):
                        c = g * 8 + i
                        nc.tensor.matmul(
                            psv8[:, i, :],
                            xt_sb[:, p, c * 128:(c + 1) * 128],
                            wv_sb[:, ws],
                            start=True, stop=True)
                    dst = v_sb[:, p, g * 8:(g + 1) * 8, :].rearrange(
                        "p c (h e) -> p c h e", e=65)[:, :, :, 0:64]
                    src = psv[:, 0:1024].rearrange(
                        "p (c h e) -> p c h e", h=2, e=64)
                    nc.vector.tensor_copy(dst, src)

                # ---- epilogue for pair p as staged thunks ----
                def emit_epilogue_stages(p, pots, reuse_pb=False):
                    """reuse_pb: the tail variant claims the proj region once
                    (s_bcast_mm) and the shift writes its unused partitions
                    64-127 — later tail claims of the proj region must not
                    create a WAR cycle through the y-group casts."""
                    t = ep_tiles()

                    def s_sums():
                        # softmax sums (f32) + unnormalized outputs to SBUF;
                        # pots are fully released after these reads.
                        for a, pc in ((0, t["pca"]), (1, t["pcb"])):
                            nc.vector.tensor_copy(
                                t["sums"][0:1, a * QS:(a + 1) * QS],
                                pots[a][64:65, :])
                            nc.vector.tensor_copy(pc[:], pots[a][0:64, :])

                    def s_ln():
                        # 1/Z as exp(-ln Z): both anchors live in the
                        # natural_log_exp_and_others table set (no reload),
                        # and [1,1024]-row transcendentals are cheapest on
                        # ACT (~1.15us each; DVE/Pool lack a usable divide).
                        nc.scalar.activation(t["lnrow"][0:1, :],
                                             t["sums"][0:1, :],
                                             mybir.ActivationFunctionType.Ln)

                    def s_recip():
                        nc.scalar.activation(t["recipb"][0:1, :],
                                             t["lnrow"][0:1, :],
                                             Exp, scale=-1.0)

                    def s_bcast_mm():
                        pb = t["pb"] = proj(f"ep{p}")
                        for a in range(2):
                            nc.tensor.matmul(
                                pb[0:64, a * QS:(a + 1) * QS],
                                ones_bf[0:1, 0:64],
                                t["recipb"][0:1, a * QS:(a + 1) * QS],
                                start=True, stop=True)

                    def s_bcast_cp():
                        nc.vector.tensor_copy(t["bcast"][:],
                                              t["pb"][0:64, 0:2 * QS])

                    def s_mul():
                        nc.vector.tensor_mul(otn[p][0:64, :], t["pca"][:],
                                             t["bcast"][:, 0:QS])
                        nc.vector.tensor_mul(t["tmpb"][:], t["pcb"][:],
                                             t["bcast"][:, QS:2 * QS])

                    def s_shift():
                        if reuse_pb:
                            ps2 = t["ps2"] = t["pb"]
                        else:
                            ps2 = t["ps2"] = proj(f"sh{p}")
                        nc.tensor.matmul(ps2[64:128, 0:QS], id_sb[:],
                                         t["tmpb"][:], start=True, stop=True,
                                         tile_position=(0, 64))

                    def s_ocp():
                        nc.vector.tensor_copy(otn[p][64:128, :],
                                              t["ps2"][64:128, 0:QS])

                    return [s_sums, s_ln, s_recip, s_bcast_mm, s_bcast_cp,
                            s_mul, s_shift, s_ocp]

                prev_ep = {"p": None, "pots": None}

                def emit_attn(p, fillers):
                    """fillers: dict chunk_idx -> list of thunks emitted after
                    that chunk's exp. OT emission is deferred (one chunk at
                    c2+, the leftover doubled at c9) so single-buffered pots
                    never block the in-order PE queue: the pot WAR (prev
                    pair's s_sums/pot copies) resolves by chunk ~1.5."""
                    pots = [pot_pool.tile([65, QS], _f32, tag=f"pot{a}",
                                          name=f"pot{p}_{a}") for a in range(2)]
                    ats_q = []
                    emitted = {"n": 0}

                    def emit_ot():
                        c = emitted["n"]
                        emitted["n"] += 1
                        at = ats_q.pop(0)
                        for a in range(2):
                            nc.tensor.matmul(
                                pots[a][:],
                                v_sb[:, p, c, 65 * a:65 * a + 65],
                                at[:, a * QS:(a + 1) * QS],
                                start=(c == 0), stop=(c == NKC - 1))

                    # chunk -> number of OTs to emit after that chunk's exp
                    ot_sched = {2: 1, 3: 1, 4: 1, 5: 1, 6: 1, 7: 1, 8: 1,
                                9: 2}

                    for c in range(NKC):
                        pss = slot(f"pss{p}_{c}")
                        for a in range(2):
                            r = slice(64 * a, 64 * a + 64)
                            nc.tensor.matmul(
                                pss[:, a * QS:(a + 1) * QS],
                                kt_sb[r, p, c * 128:(c + 1) * 128],
                                qt_sb[r, p, :],
                                start=True, stop=True)
                        at = attn_pool.tile([128, 2 * QS], _bf, tag="at")
                        nc.scalar.activation(at[:], pss[:], Exp, scale=SCALE)
                        ats_q.append(at)
                        for thunk in fillers.get(c, ()):
                            thunk()
                        if c >= 10:
                            emit_ot()
                        else:
                            for _ in range(ot_sched.get(c, 0)):
                                emit_ot()
                    prev_ep["p"], prev_ep["pots"] = p, pots
                    # OT(15) deferred to the next pair's chunk 0 so the next
                    # scores/exp start before it in PE order
                    return emit_ot

                # software pipeline: kt(0)g0 (proj region) and qt(0) (slot
                # region, so the two chains don't serialize through the
                # single proj bank while DMAs land) run upfront; proj(p+1)
                # and the epilogue(p-1) interleave into attn(p). The extra
                # slot claim pairs with the warmup claim, keeping the
                # scores-slot parity intact.
                emit_qt(0, region=slot("qt0s"))
                emit_kt(0, 0, split=True)
                emit_v(0, 0)
                pending_ot = None
                for p in range(NPAIR):
                    fillers = {}

                    def put(c, thunk):
                        fillers.setdefault(c, []).append(thunk)

                    if pending_ot is not None:
                        put(0, pending_ot)
                    # V(p,0) was produced one pair ahead (@13 below) so OTs
                    # can start at c2; the second half lands here at c1
                    # (cast done ~c2.3, consumed from OT(p,8) ~c9)
                    put(1, lambda p=p: emit_v(p, 1))
                    if p == 0:
                        put(3, lambda: emit_kt(0, 1))
                        put(5, lambda: emit_qt(1))
                        put(7, lambda: emit_kt(1, 0))
                        put(9, lambda: emit_v(1, 0))
                        put(11, lambda: emit_kt(1, 1))
                    if prev_ep["pots"] is not None:
                        stages = emit_epilogue_stages(prev_ep["p"],
                                                      prev_ep["pots"])
                        # sums@0 (releases pots); Ln@1/Exp@2 slot into the
                        # ACT FIFO (recipb lands ~c4.3, so the bcast matmul
                        # waits until c5); then the mul/shift tail
                        for c, s in zip((0, 1, 2, 5, 6, 6, 7, 8), stages):
                            put(c, s)
                    if p >= 1 and p + 1 < NPAIR:
                        q = p + 1
                        put(9, lambda q=q: emit_qt(q))
                        put(11, lambda q=q: emit_kt(q, 0))
                        put(13, lambda q=q: emit_v(q, 0))
                        put(15, lambda q=q: emit_kt(q, 1))
                    pending_ot = emit_attn(p, fillers)
                pending_ot()

                groups = [(j, dh) for j in range(QS // 128) for dh in range(2)]
                pys = {}

                def y_prefix(g, py, npre=NPAIR - 1):
                    j, dh = g
                    dsl = slice(dh * 512, (dh + 1) * 512)
                    pys[g] = py
                    for k in range(npre):
                        nc.tensor.matmul(
                            py[:, 0:512],
                            otn[k][:, j * 128:(j + 1) * 128],
                            wo_sb[:, k, dsl],
                            start=(k == 0), stop=False)

                def y_finish(g):
                    j, dh = g
                    dsl = slice(dh * 512, (dh + 1) * 512)
                    py = pys[g]
                    nc.tensor.matmul(
                        py[:, 0:512],
                        otn[NPAIR - 1][:, j * 128:(j + 1) * 128],
                        wo_sb[:, NPAIR - 1, dsl],
                        start=False, stop=False)
                    nc.tensor.matmul(py[:, 0:512], ones_bf[0:1, :],
                                     bo_sb[0:1, dsl],
                                     start=False, stop=True)
                    ysb = y_pool.tile([128, 512], _bf, tag="ysb")
                    nc.vector.tensor_copy(ysb[:], py[:, 0:512])
                    nc.sync.dma_start(
                        out_d[j * 128:(j + 1) * 128, dsl], ysb[:])

                # tail: every PSUM resource is free once the last exp has
                # read its slot, so ALL 8 y groups accumulate concurrently
                # (2 slot bufs + the proj region give 6 bank-halves; the two
                # pot banks are reclaimed as [128,512] tiles for the rest).
                # 56 back-to-back prefix matmuls hide the whole epilogue
                # chain and keep the PE clock warm through the transition.
                stages = emit_epilogue_stages(prev_ep["p"], prev_ep["pots"],
                                              reuse_pb=True)
                s_sums, s_ln, s_recip, s_bcast_mm, s_bcast_cp, s_mul, \
                    s_shift, s_ocp = stages
                s_sums(); s_ln(); s_recip()
                yA = slot("yA")
                yB = slot("yB")
                yD = pot_pool.tile([128, QS], _f32, tag="pot0", name="yD")
                yE = pot_pool.tile([128, QS], _f32, tag="pot1", name="yE")
                y_prefix(groups[0], yA[:, 0:512])
                y_prefix(groups[1], yA[:, 512:1024])
                s_bcast_mm()          # claims the proj region (shift reuses)
                y_prefix(groups[2], yB[:, 0:512])
                y_prefix(groups[3], yB[:, 512:1024])
                s_bcast_cp(); s_mul(); s_shift(); s_ocp()
                # finishes interleave with the remaining prefixes so the
                # output casts + DMAs overlap the matmul stream instead of
                # serializing at the very end
                y_prefix(groups[4], yD[:, 0:512])
                y_finish(groups[0])
                y_prefix(groups[5], yE[:, 0:512])
                y_finish(groups[1])
                # the proj region frees once ocp has read the shift output;
                # claim it last for the final two groups
                yC = proj_pool.tile([128, 1024], _f32, tag="proj", name="yC")
                y_prefix(groups[6], yC[:, 0:512])
                y_finish(groups[2])
                y_prefix(groups[7], yC[:, 512:1024])
                for g in groups[3:]:
                    y_finish(g)

    _split_excess_waits(nc, 1)
    return nc


def _blockdiag_pack(w):
    """[H, HD, HD] -> [128, NPAIR*128] blockdiagonal per pair, k-major."""
    out = np.zeros((128, NPAIR * 128), np.float32)
    for p in range(NPAIR):
        out[0:64, p * 128 + 0:p * 128 + 64] = w[2 * p]
        out[64:128, p * 128 + 64:p * 128 + 128] = w[2 * p + 1]
    return out.astype(BF16)


def _bias_pack(b):
    """[H, HD] -> [128, NPAIR] (pair bias along partitions)."""
    out = np.zeros((128, NPAIR), np.float32)
    for p in range(NPAIR):
        out[0:64, p] = b[2 * p]
        out[64:128, p] = b[2 * p + 1]
    return out


def prepare_inputs(X, Wq, bq, Wk, bk, Wv, bv, Wo, bo):
    """Host-side shard + pack. Returns in_maps (one dict per core).

    bk is accepted but unused: a per-query constant added to every key's
    score cancels exactly in softmax."""
    X = np.asarray(X, np.float32)
    Wo = np.asarray(Wo, np.float32)
    # fold the V bias through the output projection: bo2 = bv @ Wo + bo
    bo2 = (np.asarray(bv, np.float32).reshape(-1) @ Wo
           + np.asarray(bo, np.float32))
    common = {
        "wk": _blockdiag_pack(np.asarray(Wk, np.float32)),
        "wq": _blockdiag_pack(np.asarray(Wq, np.float32)),
        "wv": _blockdiag_pack(np.asarray(Wv, np.float32)),
        "bq": _bias_pack(np.asarray(bq, np.float32)),
        "wo": np.ascontiguousarray(
            Wo.reshape(8, 128, D).transpose(1, 0, 2)
        ).astype(BF16),
        "bo": bo2.reshape(1, D).astype(BF16),
        "ident": np.eye(64, dtype=np.float32).astype(BF16),
    }
    xts = []
    for b in range(B):
        xt = np.ascontiguousarray(X[b].T)                   # [D, S]
        xts.append(np.ascontiguousarray(
            xt.reshape(8, 128, S).transpose(1, 0, 2)).astype(BF16))
    in_maps = []
    for c in range(NCORES):
        b = c // (NCORES // B)
        q0 = (c % (NCORES // B)) * QS
        m = dict(common)
        m["xt"] = xts[b]
        m["xtq"] = np.ascontiguousarray(xts[b][:, :, q0:q0 + QS])
        in_maps.append(m)
    return in_maps


_NC_CACHE = None


def _get_nc():
    global _NC_CACHE
    if _NC_CACHE is None:
        _NC_CACHE = build_nc()
    return _NC_CACHE


def kernel(X, Wq, bq, Wk, bk, Wv, bv, Wo, bo):
    nc = _get_nc()
    in_maps = prepare_inputs(X, Wq, bq, Wk, bk, Wv, bv, Wo, bo)
    res = run_bass_kernel_spmd(nc, in_maps, core_ids=list(range(NCORES)))
    out = np.empty((B, S, D), np.float32)
    for c in range(NCORES):
        b = c // (NCORES // B)
        q0 = (c % (NCORES // B)) * QS
        out[b, q0:q0 + QS, :] = np.asarray(res.results[c]["out"],
                                           dtype=np.float32)
    return out
